# revision 6
# baseline (speedup 1.0000x reference)
"""LinearAttention TRN2 kernel v3: data-parallel over batch on 8 cores.

Math (same as v2):
  Wq' = per-head Wq @ P (feature map folded into the Q projection); same for K.
  QkT = relu(Wq'^T q^T + bq)    [HF, tok]   (transposed activations)
  Ksum[hf,b] = sum_s relu(Wk'^T k^T + bk)
  U^T[c,b,h] = sum_d WvT[hd,c] Ksum[hd,b]
  SrowT[v,h] = sum_c value[b,v,c] U^T[c,b,h]
  Z[tok] = per-head column sums of QkT;  zrec = 1/Z
  d1 = QkT * Srow * zrec  (in-place over QkT, bf16)
  finT = Wo^T d1 + bo

Precision scheme (v3, rel-l2 ~0.008):
  K-proj:   single fp8e4 DoubleRow GEMM (quant noise averages out in the
            positive token-sum that produces Ksum).
  Q-proj,
  out-proj: single-pass bf16. On real TRN2 the PE streams 1 col/cycle for
            both bf16 (128-row) and fp8-DR (256-row) matmuls, so bf16
            single-pass (16 MM/tile) beats 3-term compensated fp8
            (24 MM/tile) by 1.5x at better accuracy.
  V path:   bf16.

Pipeline (per core: BL=8 batches, M=2048 tokens, strips of 512 tokens):
  B : K-proj, t-outer over 4 strips (fp8 DR).  Prefetch xq01, wvt, ut loads.
  QA: Q-proj strips 0,1 t-outer (wq read once for both strips).
      Interleaved: U^T chunks (t<8), srow per batch (t>=8), xq23 prefetch.
  QB: Q-proj strips 2,3. Interleaved: scale blocks for strips 0,1.
  OA: out-proj strips 0,1 (wo read once for pair).
      Interleaved: leftover strip-1 blocks + scale blocks for strips 2,3.
  OB: out-proj strips 2,3.
Weight traffic: wq and wo are each read twice (once per strip-pair);
total HBM traffic ~72MB/core vs ~590us of PE work -> DMA stays hidden.
"""
import numpy as np
import ml_dtypes

B, S, D, H = 64, 256, 2048, 8
DK = D // H
F = 256
EPS = 1e-8
NCORES = 8
BL = B // NCORES          # 8 batches per core
M = BL * S                # 2048 tokens per core
KT = D // 128             # 16 k-tiles
NSTRIP = 4                # strips of 512 tokens (2 batches)
SW = M // NSTRIP          # 512

SXK, SWK = 4.0, 16.0      # host pre-scales for fp8 K-path operands


def _build(rep=1):
    import concourse.bass as bass
    import concourse.mybir as mybir
    import concourse.tile as tile_mod
    from concourse.vector_clock import ScopedClock

    # ---- workaround: this walrus build allows ONE sync wait per instruction.
    if not getattr(tile_mod, "_onewait_patched", False):
        _orig_add = tile_mod.TileContext._add_instruction

        def _patched_add(self, inst):
            si = inst.sync_info
            if si is not None and si.on_wait is not None and len(si.on_wait) > 1:
                waits = list(si.on_wait)
                for w in waits[:-1]:
                    nop = mybir.InstNoOp(name=self.nc.get_next_instruction_name())
                    nop.engine = inst.engine
                    nop.sync_info = mybir.SyncInfo(on_wait=[w], on_update=[])
                    _orig_add(self, nop)
                inst.sync_info = mybir.SyncInfo(
                    on_wait=[waits[-1]], on_update=list(si.on_update)
                )
            _orig_add(self, inst)

        def _patched_drain(self, tick_clock, wait_clock):
            gc = tick_clock.global_clock
            items = gc.items() if hasattr(gc, "items") else [(None, gc)]
            for scope, vc in items:
                for proc in range(len(vc)):
                    t = vc[proc]
                    if t > 0:
                        nop = self.nc.sync.nop()
                        req = ScopedClock()
                        req.require_at_least(scope, proc, t)
                        wait_clock.add_sem_waits(nop.ins, req)
            self.nc.sync.drain()
            self.nc.all_engine_barrier()
            popped = self.nc._tile_sem_poison_stack.pop()
            assert popped is self._sem_poison
            self.nc.clear_and_free_semaphores(list(self.sems.allocated().values()))
            self.nc.all_engine_barrier()

        tile_mod.TileContext._add_instruction = _patched_add
        tile_mod.TileContext._drain_and_barrier = _patched_drain
        tile_mod._onewait_patched = True

    f32 = mybir.dt.float32
    bf16 = mybir.dt.bfloat16
    fp8 = mybir.dt.float8e4
    Relu = mybir.ActivationFunctionType.Relu
    Alu = mybir.AluOpType
    DR = mybir.MatmulPerfMode.DoubleRow

    nc = bass.Bass()
    xq = nc.declare_dram_parameter("xq", [D, M], bf16, isOutput=False)
    xk8 = nc.declare_dram_parameter("xk8", [D, M], fp8, isOutput=False)
    xv = nc.declare_dram_parameter("xv", [D, M], bf16, isOutput=False)
    wqb = nc.declare_dram_parameter("wqb", [KT, 128, D], bf16, isOutput=False)
    wk8 = nc.declare_dram_parameter("wk8", [KT, 128, D], fp8, isOutput=False)
    wvt = nc.declare_dram_parameter("wvt", [KT, 128, D], bf16, isOutput=False)
    wob = nc.declare_dram_parameter("wob", [KT, 128, D], bf16, isOutput=False)
    sosel = nc.declare_dram_parameter("sosel", [8, H * 128], bf16, isOutput=False)
    ident = nc.declare_dram_parameter("ident", [8, 8], bf16, isOutput=False)
    bqp = nc.declare_dram_parameter("bqp", [D], f32, isOutput=False)
    bkp = nc.declare_dram_parameter("bkp", [D], f32, isOutput=False)
    bob = nc.declare_dram_parameter("bob", [D], f32, isOutput=False)
    fin = nc.declare_dram_parameter("fin", [D, M], bf16, isOutput=True)

    def r128(t):
        return t.rearrange("(t p) m -> p t m", p=128)

    with tile_mod.TileContext(nc) as tc:
        with (
            nc.allow_low_precision(reason="fp8/bf16 pipeline by design"),
            tc.tile_pool(name="persist", bufs=1) as ppool,
            tc.tile_pool(name="wchunk", bufs=2) as wcpool,      # wk chunks (B)
            tc.tile_pool(name="wqpool", bufs=3) as wqpool,      # wq/wo chunks
            tc.tile_pool(name="wvpool", bufs=3) as wvpool,      # wvt chunks
            tc.tile_pool(name="xkpool", bufs=4) as xkpool,      # xk strips (B)
            tc.tile_pool(name="xvpool", bufs=2) as xvpool,      # xv per batch
            tc.tile_pool(name="xqpool", bufs=3) as xqpool,      # xq strips
            tc.tile_pool(name="qkpool", bufs=4) as qkpool,      # qk strips
            tc.tile_pool(name="scpool", bufs=2) as scpool,      # B epilogue scrap
            tc.tile_pool(name="zpool", bufs=3) as zpool,        # zr tiles
            tc.tile_pool(name="psbig", bufs=4, space="PSUM") as psbig,
            tc.tile_pool(name="psbpool", bufs=2, space="PSUM") as psbpool,
            tc.tile_pool(name="pssmall", bufs=2, space="PSUM") as pssmall,
        ):
            # persistent constants / small state
            bq_sb = ppool.tile([128, KT], f32, tag="bq")
            bk_sb = ppool.tile([128, KT], f32, tag="bk")
            bo_sb = ppool.tile([128, KT], f32, tag="bo")

            # zind[:, 7-h:15-h] is a [128,8] matrix whose column h is ones
            zind = ppool.tile([128, 15], bf16, tag="zind")
            nc.vector.memset(zind[:], 0.0)
            nc.vector.memset(zind[:, 7:8], 1.0)
            # so_sel[:, h, :]: [8,128] selector, row h = 1.0 (broadcast head h)
            so_sel = ppool.tile([8, H, 128], bf16, tag="sosel")
            nc.sync.dma_start(so_sel[:], sosel[:, :].rearrange("p (h c) -> p h c", h=H))

            ident8 = ppool.tile([8, 8], bf16, tag="ident8")
            nc.sync.dma_start(ident8[:], ident[:])

            ksum = ppool.tile([128, KT, BL], f32, tag="ksum")
            ksum_bf = ppool.tile([128, KT, BL], bf16, tag="ksumbf")
            ut_sb = ppool.tile([128, KT, 64], bf16, tag="ut")
            srow = ppool.tile([128, 2, BL, H], f32, tag="srow")

            for r in range(rep):
                xq_pre = {}
                wv_tiles = {}

                def xq_load(n):
                    xqs = xqpool.tile([128, KT, SW], bf16, tag="xq")
                    nc.sync.dma_start(xqs[:], r128(xq)[:, :, n * SW:(n + 1) * SW])
                    xq_pre[n] = xqs

                def ut_load(ct):
                    if ct >= KT or ct in wv_tiles:
                        return
                    wv_c = wvpool.tile([128, KT, 128], bf16, tag="wvp")
                    nc.sync.dma_start(wv_c[:], wvt[ct])
                    wv_tiles[ct] = wv_c

                # ---------------- phase B: K-proj -> Ksum (fp8 DR) ---------
                # t-outer: wk streamed once; all 4 xk strips resident.
                xss = []
                wk_pre = []
                for n in range(NSTRIP):
                    xs = xkpool.tile([128, KT, SW], fp8, tag="xk")
                    nc.sync.dma_start(xs[:], r128(xk8)[:, :, n * SW:(n + 1) * SW])
                    xss.append(xs)
                    if n < 1:
                        wc = wcpool.tile([128, KT, 128], fp8, tag="wc8")
                        nc.sync.dma_start(wc[:], wk8[n])
                        wk_pre.append(wc)
                    if n == 3 and r == 0:
                        nc.sync.dma_start(bq_sb[:], bqp.rearrange("(t p) -> p t", p=128))
                        nc.sync.dma_start(bk_sb[:], bkp.rearrange("(t p) -> p t", p=128))
                        nc.sync.dma_start(bo_sb[:], bob.rearrange("(t p) -> p t", p=128))
                for t in range(KT):
                    if t < 1:
                        wk_c = wk_pre[t]
                    else:
                        wk_c = wcpool.tile([128, KT, 128], fp8, tag="wc8")
                        nc.sync.dma_start(wk_c[:], wk8[t])
                    # prefetch xq strips 0,1 + first wvt chunks during B
                    if t in (3, 9):
                        xq_load(0 if t == 3 else 1)
                    if t >= 12:
                        ut_load(t - 12)              # ct = 0..3
                    for n in range(NSTRIP):
                        ps = psbig.tile([128, SW], f32, tag="big")
                        for j in range(8):
                            nc.tensor.matmul(ps[:], wk_c[:, 2 * j:2 * j + 2, :],
                                             xss[n][:, 2 * j:2 * j + 2, :],
                                             start=(j == 0), stop=(j == 7),
                                             perf_mode=DR)
                        scrap = scpool.tile([128, 256, 2], bf16, tag="d1")
                        for half in range(2):
                            b = 2 * n + half
                            nc.scalar.activation(
                                scrap[:, :, half], ps[:, half * 256:(half + 1) * 256],
                                Relu, bias=bk_sb[:, t:t + 1], scale=1.0 / (SXK * SWK),
                                accum_out=ksum[:, t, b:b + 1])
                nc.vector.tensor_scalar(ksum_bf[:], ksum[:], S * EPS, None, Alu.add)

                # ---- U^T chunk worker: full-ct chunks (128-partition psu) ----
                def ut_mm(ct):
                    wv_c = wv_tiles.pop(ct)
                    psu = pssmall.tile([128, 64], f32, tag="small")
                    for h in range(H):
                        for j in range(2):
                            t = 2 * h + j
                            nc.tensor.matmul(psu[:, h * 8:(h + 1) * 8],
                                             wv_c[:, t, :], ksum_bf[:, t, :],
                                             start=(j == 0), stop=(j == 1))
                    nc.vector.tensor_copy(ut_sb[:, ct, :], psu[:])

                srow_pending = {}

                def srow_load(b):
                    xsv = xvpool.tile([128, KT, S], bf16, tag="xsv", bufs=2)
                    nc.sync.dma_start(xsv[:], r128(xv)[:, :, b * S:(b + 1) * S])
                    srow_pending[b] = xsv

                def srow_mms(b):
                    xsv = srow_pending.pop(b)
                    pss = pssmall.tile([8, S], f32, tag="small")
                    for ct in range(KT):
                        nc.tensor.matmul(pss[:], ut_sb[:, ct, b::8],
                                         xsv[:, ct, :],
                                         start=(ct == 0), stop=(ct == KT - 1))
                    sh = scpool.tile([8, S], bf16, tag="srhb")
                    nc.vector.tensor_copy(sh[:], pss[:])
                    for vch in range(2):
                        psT = pssmall.tile([128, 8], bf16, tag="small")
                        nc.tensor.transpose(psT[:], sh[:, vch * 128:(vch + 1) * 128],
                                            ident8[:])
                        nc.vector.tensor_copy(srow[:, vch, b, :], psT[:])

                # ---- Q-proj for a strip pair, t-outer (bf16 single pass) ----
                def q_pair(n0, n1, per_t=None):
                    qks = {n: qkpool.tile([128, KT, SW], bf16, tag="qk",
                                          name=f"qk{n}")
                           for n in (n0, n1)}
                    for t in range(KT):
                        if per_t is not None:
                            per_t(t)
                        wqc = wqpool.tile([128, KT, 128], bf16, tag="wqc")
                        nc.sync.dma_start(wqc[:], wqb[t])
                        for n in (n0, n1):
                            ps = psbig.tile([128, SW], f32, tag="big")
                            for j in range(KT):
                                nc.tensor.matmul(ps[:], wqc[:, j, :],
                                                 xq_pre[n][:, j, :],
                                                 start=(j == 0), stop=(j == KT - 1))
                            nc.scalar.activation(qks[n][:, t, :], ps[:], Relu,
                                                 bias=bq_sb[:, t:t + 1], scale=1.0)
                    return qks

                # ---- scale blocks for one strip: Z, then per-head d1 (in
                # place over qk). block(i) for i=0..8.
                def scale_blocks(n, qk):
                    state = {}

                    def block(i):
                        if i == 0:
                            pszall = pssmall.tile([8, SW], f32, tag="small")
                            for t in range(KT):
                                h = t // 2
                                nc.tensor.matmul(pszall[:], zind[:, 7 - h:15 - h],
                                                 qk[:, t, :],
                                                 start=(t == 0), stop=(t == KT - 1))
                            zrall = zpool.tile([8, SW], bf16, tag="zr")
                            nc.vector.reciprocal(zrall[:], pszall[:])
                            state['zr'] = zrall
                            return
                        h = i - 1
                        psb = psbpool.tile([128, SW], f32, tag="psb")
                        nc.tensor.matmul(psb[:], so_sel[:, h, :], state['zr'][:],
                                         start=True, stop=True)
                        for fh in range(2):
                            t = 2 * h + fh
                            for half in range(2):
                                b = 2 * n + half
                                sl = slice(half * 256, (half + 1) * 256)
                                nc.vector.scalar_tensor_tensor(
                                    qk[:, t, sl], qk[:, t, sl],
                                    srow[:, fh, b, h:h + 1], psb[:, sl],
                                    Alu.mult, Alu.mult)

                    return block

                # ---- out-proj for a strip pair, m-outer (bf16) ----
                def o_pair(strips, prefetched=None, per_m=None):
                    for m in range(KT):
                        if per_m is not None:
                            per_m(m)
                        if m == 0 and prefetched is not None:
                            wo_c = prefetched
                        else:
                            wo_c = wqpool.tile([128, KT, 128], bf16, tag="wqc")
                            nc.sync.dma_start(wo_c[:], wob[m])
                        for n, dt in strips:
                            ps = psbig.tile([128, SW], f32, tag="big")
                            for j in range(KT):
                                nc.tensor.matmul(ps[:], wo_c[:, j, :],
                                                 dt[:, j, :],
                                                 start=(j == 0), stop=(j == KT - 1))
                            fo = scpool.tile([128, SW], bf16, tag="d1")
                            nc.vector.tensor_scalar(fo[:], ps[:], 1.0,
                                                    bo_sb[:, m:m + 1], Alu.mult, Alu.add)
                            nc.sync.dma_start(
                                fin[m * 128:(m + 1) * 128, n * SW:(n + 1) * SW], fo[:])

                def o_prefetch():
                    wo_c = wqpool.tile([128, KT, 128], bf16, tag="wqc")
                    nc.sync.dma_start(wo_c[:], wob[0])
                    return wo_c

                # ---------------- phase QA: strips 0,1 ---------------------
                def per_t_qa(t):
                    if t < 8:
                        ut_mm(2 * t)
                        ut_mm(2 * t + 1)
                        ut_load(2 * t + 4)
                        ut_load(2 * t + 5)
                    elif t < 12:
                        b = t - 8
                        srow_load(b)          # b = 0..3
                        if b > 0:
                            srow_mms(b - 1)
                    elif t == 12:
                        srow_mms(3)
                    if t == 10:
                        xq_load(2)

                qk01 = q_pair(0, 1, per_t=per_t_qa)
                # 4th xq allocation reuses strip 0's slot; must be emitted
                # after the last strip-0 reader (end of QA)
                xq_load(3)
                blk0 = scale_blocks(0, qk01[0])
                blk1 = scale_blocks(1, qk01[1])

                # ---------------- phase QB: strips 2,3 ---------------------
                # 18 blocks (blk0 x9, blk1 x9) over 16 t-slots; leftovers
                # spill into OA's per_m queue. srow for batches 4-7 also
                # runs here (needed by blk2/blk3 during OA).
                qb_sched = [(blk0, i) for i in range(9)] + \
                           [(blk1, i) for i in range(9)]

                def per_t_qb(t):
                    if t < 4:
                        srow_load(4 + t)
                        if t > 0:
                            srow_mms(3 + t)
                    elif t == 4:
                        srow_mms(7)
                    f, i = qb_sched[t]
                    f(i)

                qk23 = q_pair(2, 3, per_t=per_t_qb)
                pf = o_prefetch()
                blk2 = scale_blocks(2, qk23[2])
                blk3 = scale_blocks(3, qk23[3])

                oa_queue = qb_sched[KT:] + \
                    [(blk2, i) for i in range(9)] + \
                    [(blk3, i) for i in range(9)]
                oa_extra = len(oa_queue) - KT

                def per_m_oa(m):
                    # 20 blocks over 16 slots: run 2 on the first few slots
                    take = 2 if m < oa_extra else 1
                    for _ in range(take):
                        if oa_queue:
                            f, i = oa_queue.pop(0)
                            f(i)

                o_pair([(0, qk01[0]), (1, qk01[1])], prefetched=pf,
                       per_m=per_m_oa)
                pf2 = o_prefetch()
                o_pair([(2, qk23[2]), (3, qk23[3])], prefetched=pf2)
    return nc


_NC = None


def _prep_host(query, key, value, Wq, bq, Wk, bk, Wv, bv, Wo, bo, random_proj):
    bf = ml_dtypes.bfloat16
    f8 = ml_dtypes.float8_e4m3

    def fold(Wx):
        return np.einsum('dhk,kf->dhf', Wx.reshape(D, H, DK).astype(np.float32),
                         random_proj.astype(np.float32)).reshape(D, D)

    Wqp = fold(Wq)
    Wkp = fold(Wk)
    bqp = ((bq.reshape(H, DK) @ random_proj).reshape(D)).astype(np.float32)
    bkp = ((bk.reshape(H, DK) @ random_proj).reshape(D)).astype(np.float32)

    def chunked(Wmat):
        # [t, p, k, c] layout: chunk t reads contiguous (k, c) rows per p
        return np.ascontiguousarray(
            Wmat.reshape(KT, 128, KT, 128).transpose(2, 1, 0, 3).reshape(KT, 128, D))

    wqb_m = chunked(Wqp.astype(bf))
    wk8 = chunked((Wkp * SWK).astype(f8))
    wob_m = chunked(np.asarray(Wo, np.float32).astype(bf))
    wvt_b = chunked(np.ascontiguousarray(Wv.T).astype(bf))

    sosel = np.zeros((8, H, 128), np.float32)
    for h in range(H):
        sosel[h, h, :] = 1.0
    sosel = sosel.reshape(8, H * 128).astype(bf)

    weights = {
        "sosel": sosel,
        "ident": np.eye(8, dtype=np.float32).astype(bf),
        "wqb": wqb_m,
        "wk8": np.ascontiguousarray(wk8),
        "wvt": wvt_b,
        "wob": wob_m,
        "bqp": bqp, "bkp": bkp, "bob": bo.astype(np.float32),
    }
    in_maps = []
    for c in range(NCORES):
        sl = slice(c * BL, (c + 1) * BL)
        qT = np.ascontiguousarray(query[sl].reshape(M, D).T)
        kT = np.ascontiguousarray(key[sl].reshape(M, D).T)
        vT = np.ascontiguousarray(value[sl].reshape(M, D).T)
        in_maps.append({
            "xq": qT.astype(bf),
            "xk8": (kT * SXK).astype(f8),
            "xv": vT.astype(bf),
            **weights,
        })
    return in_maps


def kernel(query, key, value, Wq, bq, Wk, bk, Wv, bv, Wo, bo, random_proj):
    global _NC
    from concourse.bass_utils import run_bass_kernel_spmd

    in_maps = _prep_host(query, key, value, Wq, bq, Wk, bk, Wv, bv, Wo, bo,
                         random_proj)
    if _NC is None:
        _NC = _build(rep=1)
    res = run_bass_kernel_spmd(_NC, in_maps, list(range(NCORES)))
    out = np.empty((B, S, D), dtype=np.float32)
    for c in range(NCORES):
        finT = res.results[c]["fin"]                      # [D, M] bf16
        out[c * BL:(c + 1) * BL] = finT.astype(np.float32).T.reshape(BL, S, D)
    kernel._last_in_maps = in_maps
    return out


# revision 7
# speedup vs baseline: 1.0578x; 1.0578x over previous
"""LinearAttention TRN2 kernel v3: data-parallel over batch on 8 cores.

Math (same as v2):
  Wq' = per-head Wq @ P (feature map folded into the Q projection); same for K.
  QkT = relu(Wq'^T q^T + bq)    [HF, tok]   (transposed activations)
  Ksum[hf,b] = sum_s relu(Wk'^T k^T + bk)
  U^T[c,b,h] = sum_d WvT[hd,c] Ksum[hd,b]
  SrowT[v,h] = sum_c value[b,v,c] U^T[c,b,h]
  Z[tok] = per-head column sums of QkT;  zrec = 1/Z
  d1 = QkT * Srow * zrec  (in-place over QkT, bf16)
  finT = Wo^T d1 + bo

Precision scheme (rel-l2 ~0.0083 vs reference):
  K-proj:   single fp8e4 DoubleRow GEMM (quant noise averages out in the
            positive token-sum that produces Ksum).
  Q-proj,
  out-proj: single-pass bf16. On real TRN2 the PE streams 1 col/cycle for
            both bf16 (128-row) and fp8-DR (256-row) matmuls, so bf16
            single-pass (16 MM/tile) beats 3-term compensated fp8
            (24 MM/tile) by 1.5x at better accuracy.
  V path:   bf16; srow computed with ut-stationary matmuls (8-col
            LDWEIGHTS) and transposed back to partition-major via PE.

Pipeline (per core: BL=8 batches, M=2048 tokens, strips of 512 tokens):
  B : K-proj, t-outer over 4 strips (fp8 DR).  Prefetch xq01, wvt, ut loads.
  QA: Q-proj strips 0,1 t-outer (wq read once for both strips).
      Interleaved: U^T chunks (t<8), srow per batch (t>=8), xq23 prefetch.
  QB: Q-proj strips 2,3. Interleaved: scale blocks for strips 0,1.
  OA: out-proj strips 0,1 (wo read once for pair).
      Interleaved: leftover strip-1 blocks + scale blocks for strips 2,3.
  OB: out-proj strips 2,3.
Weight traffic: wq and wo are each read twice (once per strip-pair);
total HBM traffic ~72MB/core vs ~590us of PE work -> DMA stays hidden.
"""
import numpy as np
import ml_dtypes

B, S, D, H = 64, 256, 2048, 8
DK = D // H
F = 256
EPS = 1e-8
NCORES = 8
BL = B // NCORES          # 8 batches per core
M = BL * S                # 2048 tokens per core
KT = D // 128             # 16 k-tiles
NSTRIP = 4                # strips of 512 tokens (2 batches)
SW = M // NSTRIP          # 512

SXK, SWK = 4.0, 16.0      # host pre-scales for fp8 K-path operands


def _build(rep=1):
    import concourse.bass as bass
    import concourse.mybir as mybir
    import concourse.tile as tile_mod
    from concourse.vector_clock import ScopedClock

    # ---- workaround: this walrus build allows ONE sync wait per instruction.
    if not getattr(tile_mod, "_onewait_patched", False):
        _orig_add = tile_mod.TileContext._add_instruction

        def _patched_add(self, inst):
            si = inst.sync_info
            if si is not None and si.on_wait is not None and len(si.on_wait) > 1:
                waits = list(si.on_wait)
                for w in waits[:-1]:
                    nop = mybir.InstNoOp(name=self.nc.get_next_instruction_name())
                    nop.engine = inst.engine
                    nop.sync_info = mybir.SyncInfo(on_wait=[w], on_update=[])
                    _orig_add(self, nop)
                inst.sync_info = mybir.SyncInfo(
                    on_wait=[waits[-1]], on_update=list(si.on_update)
                )
            _orig_add(self, inst)

        def _patched_drain(self, tick_clock, wait_clock):
            gc = tick_clock.global_clock
            items = gc.items() if hasattr(gc, "items") else [(None, gc)]
            for scope, vc in items:
                for proc in range(len(vc)):
                    t = vc[proc]
                    if t > 0:
                        nop = self.nc.sync.nop()
                        req = ScopedClock()
                        req.require_at_least(scope, proc, t)
                        wait_clock.add_sem_waits(nop.ins, req)
            self.nc.sync.drain()
            self.nc.all_engine_barrier()
            popped = self.nc._tile_sem_poison_stack.pop()
            assert popped is self._sem_poison
            self.nc.clear_and_free_semaphores(list(self.sems.allocated().values()))
            self.nc.all_engine_barrier()

        tile_mod.TileContext._add_instruction = _patched_add
        tile_mod.TileContext._drain_and_barrier = _patched_drain
        tile_mod._onewait_patched = True

    f32 = mybir.dt.float32
    bf16 = mybir.dt.bfloat16
    fp8 = mybir.dt.float8e4
    Relu = mybir.ActivationFunctionType.Relu
    Alu = mybir.AluOpType
    DR = mybir.MatmulPerfMode.DoubleRow

    nc = bass.Bass()
    xq = nc.declare_dram_parameter("xq", [D, M], bf16, isOutput=False)
    xk8 = nc.declare_dram_parameter("xk8", [D, M], fp8, isOutput=False)
    xv = nc.declare_dram_parameter("xv", [D, M], bf16, isOutput=False)
    wqb = nc.declare_dram_parameter("wqb", [KT, 128, D], bf16, isOutput=False)
    wk8 = nc.declare_dram_parameter("wk8", [KT, 128, D], fp8, isOutput=False)
    wvt = nc.declare_dram_parameter("wvt", [KT, 128, D], bf16, isOutput=False)
    wob = nc.declare_dram_parameter("wob", [KT, 128, D], bf16, isOutput=False)
    sosel = nc.declare_dram_parameter("sosel", [8, H * 128], bf16, isOutput=False)
    ident = nc.declare_dram_parameter("ident", [8, 8], bf16, isOutput=False)
    bqp = nc.declare_dram_parameter("bqp", [D], f32, isOutput=False)
    bkp = nc.declare_dram_parameter("bkp", [D], f32, isOutput=False)
    bob = nc.declare_dram_parameter("bob", [D], f32, isOutput=False)
    fin = nc.declare_dram_parameter("fin", [D, M], bf16, isOutput=True)

    def r128(t):
        return t.rearrange("(t p) m -> p t m", p=128)

    with tile_mod.TileContext(nc) as tc:
        with (
            nc.allow_low_precision(reason="fp8/bf16 pipeline by design"),
            tc.tile_pool(name="persist", bufs=1) as ppool,
            tc.tile_pool(name="wchunk", bufs=2) as wcpool,      # wk chunks (B)
            tc.tile_pool(name="wqpool", bufs=3) as wqpool,      # wq/wo chunks
            tc.tile_pool(name="wvpool", bufs=3) as wvpool,      # wvt chunks
            tc.tile_pool(name="xkpool", bufs=4) as xkpool,      # xk strips (B)
            tc.tile_pool(name="xvpool", bufs=2) as xvpool,      # xv per batch
            tc.tile_pool(name="xqpool", bufs=3) as xqpool,      # xq strips
            tc.tile_pool(name="qkpool", bufs=4) as qkpool,      # qk strips
            tc.tile_pool(name="scpool", bufs=2) as scpool,      # B epilogue scrap
            tc.tile_pool(name="zpool", bufs=3) as zpool,        # zr tiles
            tc.tile_pool(name="psbig", bufs=4, space="PSUM") as psbig,
            tc.tile_pool(name="psbpool", bufs=2, space="PSUM") as psbpool,
            tc.tile_pool(name="pssmall", bufs=2, space="PSUM") as pssmall,
        ):
            # persistent constants / small state
            bq_sb = ppool.tile([128, KT], f32, tag="bq")
            bk_sb = ppool.tile([128, KT], f32, tag="bk")
            bo_sb = ppool.tile([128, KT], f32, tag="bo")

            # zind[:, 7-h:15-h] is a [128,8] matrix whose column h is ones
            zind = ppool.tile([128, 15], bf16, tag="zind")
            nc.vector.memset(zind[:], 0.0)
            nc.vector.memset(zind[:, 7:8], 1.0)
            # so_sel[:, h, :]: [8,128] selector, row h = 1.0 (broadcast head h)
            so_sel = ppool.tile([8, H, 128], bf16, tag="sosel")
            nc.sync.dma_start(so_sel[:], sosel[:, :].rearrange("p (h c) -> p h c", h=H))

            ident8 = ppool.tile([8, 8], bf16, tag="ident8")
            nc.sync.dma_start(ident8[:], ident[:])

            ksum = ppool.tile([128, KT, BL], f32, tag="ksum")
            ksum_bf = ppool.tile([128, KT, BL], bf16, tag="ksumbf")
            ut_sb = ppool.tile([128, KT, 64], bf16, tag="ut")
            srow = ppool.tile([128, 2, BL, H], f32, tag="srow")

            for r in range(rep):
                xq_pre = {}
                wv_tiles = {}

                def xq_load(n):
                    xqs = xqpool.tile([128, KT, SW], bf16, tag="xq")
                    nc.sync.dma_start(xqs[:], r128(xq)[:, :, n * SW:(n + 1) * SW])
                    xq_pre[n] = xqs

                def ut_load(ct):
                    if ct >= KT or ct in wv_tiles:
                        return
                    wv_c = wvpool.tile([128, KT, 128], bf16, tag="wvp")
                    nc.sync.dma_start(wv_c[:], wvt[ct])
                    wv_tiles[ct] = wv_c

                # ---------------- phase B: K-proj -> Ksum (fp8 DR) ---------
                # t-outer: wk streamed once; all 4 xk strips resident.
                xss = []
                wk_pre = []
                for n in range(NSTRIP):
                    xs = xkpool.tile([128, KT, SW], fp8, tag="xk")
                    nc.sync.dma_start(xs[:], r128(xk8)[:, :, n * SW:(n + 1) * SW])
                    xss.append(xs)
                    if n < 1:
                        wc = wcpool.tile([128, KT, 128], fp8, tag="wc8")
                        nc.sync.dma_start(wc[:], wk8[n])
                        wk_pre.append(wc)
                    if n == 3 and r == 0:
                        nc.sync.dma_start(bq_sb[:], bqp.rearrange("(t p) -> p t", p=128))
                        nc.sync.dma_start(bk_sb[:], bkp.rearrange("(t p) -> p t", p=128))
                        nc.sync.dma_start(bo_sb[:], bob.rearrange("(t p) -> p t", p=128))
                for t in range(KT):
                    if t < 1:
                        wk_c = wk_pre[t]
                    else:
                        wk_c = wcpool.tile([128, KT, 128], fp8, tag="wc8")
                        nc.sync.dma_start(wk_c[:], wk8[t])
                    # prefetch xq strips 0,1 + first wvt chunks during B
                    if t in (3, 9):
                        xq_load(0 if t == 3 else 1)
                    if t >= 12:
                        ut_load(t - 12)              # ct = 0..3
                    for n in range(NSTRIP):
                        ps = psbig.tile([128, SW], f32, tag="big")
                        for j in range(8):
                            nc.tensor.matmul(ps[:], wk_c[:, 2 * j:2 * j + 2, :],
                                             xss[n][:, 2 * j:2 * j + 2, :],
                                             start=(j == 0), stop=(j == 7),
                                             perf_mode=DR)
                        scrap = scpool.tile([128, 256, 2], bf16, tag="d1")
                        for half in range(2):
                            b = 2 * n + half
                            nc.scalar.activation(
                                scrap[:, :, half], ps[:, half * 256:(half + 1) * 256],
                                Relu, bias=bk_sb[:, t:t + 1], scale=1.0 / (SXK * SWK),
                                accum_out=ksum[:, t, b:b + 1])
                nc.vector.tensor_scalar(ksum_bf[:], ksum[:], S * EPS, None, Alu.add)

                # ---- U^T chunk worker: full-ct chunks (128-partition psu) ----
                def ut_mm(ct):
                    wv_c = wv_tiles.pop(ct)
                    psu = pssmall.tile([128, 64], f32, tag="small")
                    for h in range(H):
                        for j in range(2):
                            t = 2 * h + j
                            nc.tensor.matmul(psu[:, h * 8:(h + 1) * 8],
                                             wv_c[:, t, :], ksum_bf[:, t, :],
                                             start=(j == 0), stop=(j == 1))
                    nc.vector.tensor_copy(ut_sb[:, ct, :], psu[:])

                srow_pending = {}

                def srow_load(b):
                    xsv = xvpool.tile([128, KT, S], bf16, tag="xsv", bufs=2)
                    nc.sync.dma_start(xsv[:], r128(xv)[:, :, b * S:(b + 1) * S])
                    srow_pending[b] = xsv

                def srow_mms(b):
                    xsv = srow_pending.pop(b)
                    pss = pssmall.tile([8, S], f32, tag="small")
                    for ct in range(KT):
                        nc.tensor.matmul(pss[:], ut_sb[:, ct, b::8],
                                         xsv[:, ct, :],
                                         start=(ct == 0), stop=(ct == KT - 1))
                    sh = scpool.tile([8, S], bf16, tag="srhb")
                    nc.vector.tensor_copy(sh[:], pss[:])
                    for vch in range(2):
                        psT = pssmall.tile([128, 8], bf16, tag="small")
                        nc.tensor.transpose(psT[:], sh[:, vch * 128:(vch + 1) * 128],
                                            ident8[:])
                        nc.vector.tensor_copy(srow[:, vch, b, :], psT[:])

                # ---- Q-proj for a strip pair, t-outer (bf16 single pass) ----
                def q_pair(n0, n1, per_t=None):
                    qks = {n: qkpool.tile([128, KT, SW], bf16, tag="qk",
                                          name=f"qk{n}")
                           for n in (n0, n1)}
                    for t in range(KT):
                        if per_t is not None:
                            per_t(t)
                        wqc = wqpool.tile([128, KT, 128], bf16, tag="wqc")
                        nc.sync.dma_start(wqc[:], wqb[t])
                        for n in (n0, n1):
                            ps = psbig.tile([128, SW], f32, tag="big")
                            for j in range(KT):
                                nc.tensor.matmul(ps[:], wqc[:, j, :],
                                                 xq_pre[n][:, j, :],
                                                 start=(j == 0), stop=(j == KT - 1))
                            nc.scalar.activation(qks[n][:, t, :], ps[:], Relu,
                                                 bias=bq_sb[:, t:t + 1], scale=1.0)
                    return qks

                # ---- scale blocks for one strip: Z, then per-head d1 (in
                # place over qk). block(i) for i=0..8.
                def scale_blocks(n, qk):
                    state = {}

                    def block(i):
                        if i == 0:
                            pszall = pssmall.tile([8, SW], f32, tag="small")
                            for t in range(KT):
                                h = t // 2
                                nc.tensor.matmul(pszall[:], zind[:, 7 - h:15 - h],
                                                 qk[:, t, :],
                                                 start=(t == 0), stop=(t == KT - 1))
                            zrall = zpool.tile([8, SW], bf16, tag="zr")
                            nc.vector.reciprocal(zrall[:], pszall[:])
                            state['zr'] = zrall
                            return
                        h = i - 1
                        psb = psbpool.tile([128, SW], f32, tag="psb")
                        nc.tensor.matmul(psb[:], so_sel[:, h, :], state['zr'][:],
                                         start=True, stop=True)
                        for fh in range(2):
                            t = 2 * h + fh
                            for half in range(2):
                                b = 2 * n + half
                                sl = slice(half * 256, (half + 1) * 256)
                                nc.vector.scalar_tensor_tensor(
                                    qk[:, t, sl], qk[:, t, sl],
                                    srow[:, fh, b, h:h + 1], psb[:, sl],
                                    Alu.mult, Alu.mult)

                    return block

                # ---- out-proj for a strip pair, m-outer (bf16) ----
                def o_pair(strips, prefetched=None, per_m=None):
                    for m in range(KT):
                        if per_m is not None:
                            per_m(m)
                        if m == 0 and prefetched is not None:
                            wo_c = prefetched
                        else:
                            wo_c = wqpool.tile([128, KT, 128], bf16, tag="wqc")
                            nc.sync.dma_start(wo_c[:], wob[m])
                        for n, dt in strips:
                            ps = psbig.tile([128, SW], f32, tag="big")
                            for j in range(KT):
                                nc.tensor.matmul(ps[:], wo_c[:, j, :],
                                                 dt[:, j, :],
                                                 start=(j == 0), stop=(j == KT - 1))
                            fo = scpool.tile([128, SW], bf16, tag="d1")
                            nc.vector.tensor_scalar(fo[:], ps[:], 1.0,
                                                    bo_sb[:, m:m + 1], Alu.mult, Alu.add)
                            nc.sync.dma_start(
                                fin[m * 128:(m + 1) * 128, n * SW:(n + 1) * SW], fo[:])

                def o_prefetch():
                    wo_c = wqpool.tile([128, KT, 128], bf16, tag="wqc")
                    nc.sync.dma_start(wo_c[:], wob[0])
                    return wo_c

                # ---------------- phase QA: strips 0,1 ---------------------
                def per_t_qa(t):
                    if t < 8:
                        ut_mm(2 * t)
                        ut_mm(2 * t + 1)
                        ut_load(2 * t + 4)
                        ut_load(2 * t + 5)
                    elif t < 12:
                        b = t - 8
                        srow_load(b)          # b = 0..3
                        if b > 0:
                            srow_mms(b - 1)
                    elif t == 12:
                        srow_mms(3)
                    if t == 10:
                        xq_load(2)

                qk01 = q_pair(0, 1, per_t=per_t_qa)
                # 4th xq allocation reuses strip 0's slot; must be emitted
                # after the last strip-0 reader (end of QA)
                xq_load(3)
                blk0 = scale_blocks(0, qk01[0])
                blk1 = scale_blocks(1, qk01[1])

                # ---------------- phase QB: strips 2,3 ---------------------
                # 18 blocks (blk0 x9, blk1 x9) over 16 t-slots; leftovers
                # spill into OA's per_m queue. srow for batches 4-7 also
                # runs here (needed by blk2/blk3 during OA).
                qb_sched = [(blk0, i) for i in range(9)] + \
                           [(blk1, i) for i in range(9)]

                def per_t_qb(t):
                    if t < 4:
                        srow_load(4 + t)
                        if t > 0:
                            srow_mms(3 + t)
                    elif t == 4:
                        srow_mms(7)
                    f, i = qb_sched[t]
                    f(i)

                qk23 = q_pair(2, 3, per_t=per_t_qb)
                pf = o_prefetch()
                blk2 = scale_blocks(2, qk23[2])
                blk3 = scale_blocks(3, qk23[3])

                oa_queue = qb_sched[KT:] + \
                    [(blk2, i) for i in range(9)] + \
                    [(blk3, i) for i in range(9)]
                oa_extra = len(oa_queue) - KT

                def per_m_oa(m):
                    # 20 blocks over 16 slots: run 2 on the first few slots
                    take = 2 if m < oa_extra else 1
                    for _ in range(take):
                        if oa_queue:
                            f, i = oa_queue.pop(0)
                            f(i)

                o_pair([(0, qk01[0]), (1, qk01[1])], prefetched=pf,
                       per_m=per_m_oa)
                pf2 = o_prefetch()
                o_pair([(2, qk23[2]), (3, qk23[3])], prefetched=pf2)
    return nc


_NC = None


def _prep_host(query, key, value, Wq, bq, Wk, bk, Wv, bv, Wo, bo, random_proj):
    bf = ml_dtypes.bfloat16
    f8 = ml_dtypes.float8_e4m3

    def fold(Wx):
        return np.einsum('dhk,kf->dhf', Wx.reshape(D, H, DK).astype(np.float32),
                         random_proj.astype(np.float32)).reshape(D, D)

    Wqp = fold(Wq)
    Wkp = fold(Wk)
    bqp = ((bq.reshape(H, DK) @ random_proj).reshape(D)).astype(np.float32)
    bkp = ((bk.reshape(H, DK) @ random_proj).reshape(D)).astype(np.float32)

    def chunked(Wmat):
        # [t, p, k, c] layout: chunk t reads contiguous (k, c) rows per p
        return np.ascontiguousarray(
            Wmat.reshape(KT, 128, KT, 128).transpose(2, 1, 0, 3).reshape(KT, 128, D))

    wqb_m = chunked(Wqp.astype(bf))
    wk8 = chunked((Wkp * SWK).astype(f8))
    wob_m = chunked(np.asarray(Wo, np.float32).astype(bf))
    wvt_b = chunked(np.ascontiguousarray(Wv.T).astype(bf))

    sosel = np.zeros((8, H, 128), np.float32)
    for h in range(H):
        sosel[h, h, :] = 1.0
    sosel = sosel.reshape(8, H * 128).astype(bf)

    weights = {
        "sosel": sosel,
        "ident": np.eye(8, dtype=np.float32).astype(bf),
        "wqb": wqb_m,
        "wk8": np.ascontiguousarray(wk8),
        "wvt": wvt_b,
        "wob": wob_m,
        "bqp": bqp, "bkp": bkp, "bob": bo.astype(np.float32),
    }
    in_maps = []
    for c in range(NCORES):
        sl = slice(c * BL, (c + 1) * BL)
        qT = np.ascontiguousarray(query[sl].reshape(M, D).T)
        kT = np.ascontiguousarray(key[sl].reshape(M, D).T)
        vT = np.ascontiguousarray(value[sl].reshape(M, D).T)
        in_maps.append({
            "xq": qT.astype(bf),
            "xk8": (kT * SXK).astype(f8),
            "xv": vT.astype(bf),
            **weights,
        })
    return in_maps


def kernel(query, key, value, Wq, bq, Wk, bk, Wv, bv, Wo, bo, random_proj):
    global _NC
    from concourse.bass_utils import run_bass_kernel_spmd

    in_maps = _prep_host(query, key, value, Wq, bq, Wk, bk, Wv, bv, Wo, bo,
                         random_proj)
    if _NC is None:
        _NC = _build(rep=1)
    res = run_bass_kernel_spmd(_NC, in_maps, list(range(NCORES)))
    out = np.empty((B, S, D), dtype=np.float32)
    for c in range(NCORES):
        finT = res.results[c]["fin"]                      # [D, M] bf16
        out[c * BL:(c + 1) * BL] = finT.astype(np.float32).T.reshape(BL, S, D)
    kernel._last_in_maps = in_maps
    return out
